# revision 9
# baseline (speedup 1.0000x reference)
"""Trainium2 Bass kernel for spatial multi-head self-attention (dense_transformer).

Module: x[2,256,64,64] -> qkv 1x1 conv -> 4-head attention over n=4096 spatial
positions -> out 1x1 conv + bias.

Sharding (8 cores): core = (batch b, head-pair hp, query-half qh of 2048
positions). Each core computes K/V for ITS 2 heads over the full 4096
positions (half the projection duplication of a batch/q-quarter split),
Q for its 2 heads over its 2048 positions, the full attention + softmax
for its (batch, head-pair, q-half), and the partial output projection
through its heads' w_out rows. The host sums the two head-pair partials
per (batch, q-half) and adds the bias - no device collectives.

Per-core structure, streaming over 32 k-tiles of 128 positions per
q-chunk round (qc of 512, 4 rounds):
  PE : scoresT[k,q] = k_tile.T @ q (the 2 heads row-packed at partitions
       0:64 / 64:128; the two half-row matmuls run CONCURRENTLY via the
       PE's row-group tiling)
  ACT: exp(scores) PSUM->SBUF bf16 (max-subtraction skipped; scores ~N(0,1)
       by construction so exp cannot overflow). A slice of each tile's
       columns is offloaded to the DVE via a Schraudolph bf16 bit-trick exp.
  PE : out += vT_aug.T @ exp_chunk; vT_aug carries a ones column so row 64
       accumulates the softmax denominator for free (stationary padded to
       128 columns for fast weight load; the extra rows are never read).
  DVE/GpSimd: normalize with reciprocal_approx_fast + partition_broadcast
       + gpsimd multiply (SBUF-only, keeps the DVE free for exp).
K-tiles are walked in PAIRS (scores pairs back-to-back, then the attn@V
matmuls chain weight loads through the PE background weight buffer).
PSUM: three rotating 2-bank score slots (shared with projection groups) +
two 1-bank attention accumulators. Projections stream just-in-time inside
round 0 (kproj/vtproj) with the input DMA pieces ordered by need-time;
normalize and the output projection are deferred into the following
round's schedule. Output is written per-qc as bf16 (host casts/sums in
fp32), spread across DMA queues.

Tail: the last round's denominators go through an ACT rowsum evict ->
DVE recip -> PE broadcast-matmul (ones-block stationary [2,128] spreads
rr[2,512] to rb[128,512] in PSUM, replacing two serialized 1us gpsimd
partition_broadcasts) -> DVE multiplies -> 2 outproj matmuls -> dual-queue
bf16 output DMA.

Exp engine split (EXP_SPLIT, tuned on HW): ACT takes ~60-70% of exp
columns, DVE the rest; round 0 gives the DVE less because it also
carries the kproj/vtproj projection evictions.
"""

import os
import sys
import types

import numpy as np

sys.path.insert(0, "/opt/trn_rl_repo")

import ml_dtypes  # noqa: E402

import concourse.bass as bass  # noqa: E402
import concourse.mybir as mybir  # noqa: E402
import concourse.tile as tile  # noqa: E402
from concourse import bacc  # noqa: E402
from concourse.bass_utils import run_bass_kernel_spmd  # noqa: E402

BF16 = mybir.dt.bfloat16
F32 = mybir.dt.float32
I16 = mybir.dt.int16

N_CORES = 8
CH = 256          # x channels
HID = 256         # qkv hidden (4 heads x 64)
H = 4             # heads
DH = 64           # dim per head
N = 4096          # spatial positions (64*64)
NQ = 2048         # query positions per core (q-half)
B = 2             # batch
SCALE = DH ** -0.5
NKT = N // 128    # 32 k-tiles
NQC = NQ // 512   # 4 q-chunks -> 4 rounds

# Schraudolph exp offload. SPLIT[r]: flat column split point S of the
# per-k-tile score block [128, 2*512] (head-major). The ACT exps columns
# [0:S] exactly; the DVE computes [S:1024] with a one-instruction
# Schraudolph bf16 bit-trick exp (rms rel err ~2% on those columns, largely
# cancelled by the shared softmax denominator).
_SP = os.environ.get("EXP_SPLIT", "704,608,576,576").split(",")
SPLIT = {r: int(_SP[r]) for r in range(4)}
LOG2E = float(np.log2(np.e))
SCH_A = 128.0 * LOG2E
SCH_B = 128.0 * (127.0 - 0.043677)


def _install_ntff_hook():
    """The image's antenv lacks axon_hooks; install it so trace=True works."""
    if "antenv.axon_hooks" in sys.modules:
        return
    try:
        mod = types.ModuleType("antenv.axon_hooks")
        mod._hook = None
        mod.set_axon_ntff_profile_hook = lambda h: setattr(mod, "_hook", h)
        mod.get_axon_ntff_profile_hook = lambda: mod._hook
        sys.modules["antenv.axon_hooks"] = mod
        import antenv
        antenv.axon_hooks = mod
        sys.path.insert(0, "/root/.axon_site/trn_agent_boot")
        from trn_boot import _ntff_profile_via_ctypes
        mod.set_axon_ntff_profile_hook(
            _ntff_profile_via_ctypes("/opt/axon/libaxon_pjrt.so")
        )
    except Exception:
        pass


def _build():
    nc = bacc.Bacc("TRN2", target_bir_lowering=False, debug=False,
                   num_devices=N_CORES)

    x_d = nc.dram_tensor("x", [CH, N], BF16, kind="ExternalInput").ap()
    # per-core weight pack [wk0 wk1 wq0 wq1 wv0 wv1 | wo]: this core's
    # head-pair slices only (128 hid columns), 1024 cols total
    wp_d = nc.dram_tensor("wp", [128, 1024], BF16, kind="ExternalInput").ap()
    out_d = nc.dram_tensor("out", [CH, NQ], BF16, kind="ExternalOutput").ap()

    with tile.TileContext(nc) as tc:
        with tc.tile_pool(name="const", bufs=1) as cst, \
             tc.tile_pool(name="scps", bufs=3, space="PSUM") as scps, \
             tc.tile_pool(name="outps", bufs=1, space="PSUM") as outps, \
             tc.tile_pool(name="expb", bufs=9) as expb, \
             tc.tile_pool(name="osb", bufs=2) as osbp, \
             tc.tile_pool(name="ntmp", bufs=2) as ntmp, \
             tc.tile_pool(name="fout", bufs=2) as foutp:

            def proj_ps(shape):
                return scps.tile(shape, F32, name="scp")

            # ---- persistent tensors ----
            wp_sb = cst.tile([128, 1024], BF16, name="wp")
            wk_sb = [wp_sb[:, c * 128:(c + 1) * 128] for c in range(2)]
            wq_sb = [wp_sb[:, 256 + c * 128:256 + (c + 1) * 128] for c in range(2)]
            wv_sb = [wp_sb[:, 512 + c * 128:512 + (c + 1) * 128] for c in range(2)]
            wo_sb = wp_sb[:, 768:1024]          # [128 hc, 256 oc]
            xb0h = [cst.tile([128, 512], BF16, name=f"xb0h{c}") for c in range(2)]
            xbch = [[cst.tile([128, 1024], BF16, name=f"xb{c}_{i}")
                     for i in range(4)] for c in range(2)]
            kch = [cst.tile([128, 512], BF16, name=f"k{n}") for n in range(8)]
            qch = [cst.tile([128, 512], BF16, name=f"q{qc}") for qc in range(NQC)]
            vtt = [cst.tile([128, 2, 128], BF16, name=f"vt{t}")
                   for t in range(NKT)]
            # tail broadcast stationary: head j's recip row lives at partition
            # 32*j (engine partition bases must be 32-aligned); ind maps row 0
            # -> rb cols 0:64, row 32 -> cols 64:128, other rows are zero so
            # the (memset-to-1.0) filler rows of rr contribute nothing
            ind = cst.tile([33, 128], BF16, name="ind")
            rrin = cst.tile([33, 512], F32, name="rrin")
            rrf = cst.tile([33, 512], F32, name="rrf")
            rrb = cst.tile([33, 512], BF16, name="rrb")

            # ---- input DMAs, need-ordered across the three queues ----
            # sync: critical weights first, then x chunk 2, then wo + x31
            nc.sync.dma_start(out=wp_sb[:, 0:512], in_=wp_d[:, 0:512])
            nc.sync.dma_start(out=wp_sb[:, 512:768], in_=wp_d[:, 512:768])
            nc.sync.dma_start(out=xbch[0][2][:], in_=x_d[0:128, 2048:3072])
            nc.sync.dma_start(out=xbch[1][2][:], in_=x_d[128:256, 2048:3072])
            nc.sync.dma_start(out=wp_sb[:, 768:1024], in_=wp_d[:, 768:1024])
            nc.sync.dma_start(out=xbch[1][3][:], in_=x_d[128:256, 3072:4096])
            # gpsimd: ind/rrin memsets first (feed the ACT table-load dummy)
            nc.gpsimd.memset(ind[:], 0.0)
            nc.gpsimd.memset(ind[0:1, 0:64], 1.0)
            nc.gpsimd.memset(ind[32:33, 64:128], 1.0)
            nc.gpsimd.memset(rrin[:], 1.0)
            nc.gpsimd.dma_start(out=xb0h[0][:], in_=x_d[0:128, 0:512])
            nc.gpsimd.dma_start(out=xbch[0][0][:], in_=x_d[0:128, 0:1024])
            nc.gpsimd.dma_start(out=xbch[0][1][:], in_=x_d[0:128, 1024:2048])
            nc.gpsimd.dma_start(out=xbch[0][3][:], in_=x_d[0:128, 3072:4096])
            # scalar: first xb0h piece, then pre-pull the exp ACT table with a
            # dummy activation (so the ~1.3us table load is off the critical
            # path of round 0's first exp), then the rest of its x chunks
            nc.scalar.dma_start(out=xb0h[1][:], in_=x_d[128:256, 0:512])
            nc.scalar.dma_start(out=xbch[1][0][:], in_=x_d[128:256, 0:1024])
            nc.scalar.activation(rrin[0:1, 0:1], ind[0:1, 0:1],
                                 mybir.ActivationFunctionType.Exp)
            nc.scalar.dma_start(out=xbch[1][1][:], in_=x_d[128:256, 1024:2048])

            # ---- projection emitters ----
            def kproj(n):
                ps = proj_ps([128, 512])
                for c in range(2):
                    rhs = (xb0h[c][:] if n == 0 else
                           xbch[c][n // 2][:, (n % 2) * 512:(n % 2 + 1) * 512])
                    nc.tensor.matmul(ps[:], lhsT=wk_sb[c][:], rhs=rhs,
                                     start=(c == 0), stop=(c == 1))
                nc.vector.tensor_copy(kch[n][:], ps[:])

            def qproj(qc):
                ps = proj_ps([128, 512])
                for c in range(2):
                    rhs = (xb0h[c][:] if qc == 0 else
                           xbch[c][qc // 2][:, (qc % 2) * 512:(qc % 2 + 1) * 512])
                    nc.tensor.matmul(ps[:], lhsT=wq_sb[c][:], rhs=rhs,
                                     start=(c == 0), stop=(c == 1))
                # qc0's eviction rides the idle ACT so it doesn't queue behind
                # kch[0]'s eviction on the DVE (both gate scores kt=0)
                if qc == 0:
                    nc.scalar.copy(qch[qc][:], ps[:])
                else:
                    nc.vector.tensor_copy(qch[qc][:], ps[:])

            def vtproj2(tp):
                ps = proj_ps([128, 256])
                for u in range(2):
                    t = 2 * tp + u
                    for c in range(2):
                        nc.tensor.matmul(
                            ps[:, u * 128:(u + 1) * 128],
                            lhsT=xbch[c][t // 8][:, (t % 8) * 128:(t % 8 + 1) * 128],
                            rhs=wv_sb[c],
                            start=(c == 0), stop=(c == 1))
                for u in range(2):
                    t = 2 * tp + u
                    nc.gpsimd.memset(vtt[t][:, :, DH:DH + 1], 1.0)
                    if u == 0:
                        nc.scalar.copy(
                            vtt[t][:, :, 0:DH],
                            ps[:, u * 128:(u + 1) * 128].rearrange(
                                "p (h d) -> p h d", d=DH))
                    else:
                        nc.vector.tensor_copy(
                            vtt[t][:, :, 0:DH],
                            ps[:, u * 128:(u + 1) * 128].rearrange(
                                "p (h d) -> p h d", d=DH))

            # ---- interleave schedules: round index -> {kt: [thunks]} ----
            # Round r == q-chunk r. Round 0 carries kproj(1..7) + vtproj JIT
            # (kproj(n) feeds scores kt=4n; vtproj2(tp) feeds attn@V kt=2tp,
            # which fires ~5 k-tiles behind scores). qproj(qc) spread out.
            sched = {r: {} for r in range(4)}
            sched_pre = {r: {} for r in range(4)}

            def add(r, kt, fn, *a):
                sched[r].setdefault(kt, []).append((fn, a))

            for n in range(1, 8):
                add(0, max(2, 4 * n - 3), kproj, n)
            for tp in range(NKT // 2):
                kt = 2 * tp + 2 if tp < 14 else (29 if tp == 14 else 30)
                add(0, kt, vtproj2, tp)
            add(0, 12, qproj, 1)
            add(1, 4, qproj, 2)
            add(2, 4, qproj, 3)

            # ---- deferred finishers (normalize / out-projection) ----
            o_tiles = {}   # qc -> [128, 512] bf16 (both heads stacked)
            norm_state = {}

            def get_o(qc):
                if qc not in o_tiles:
                    o_tiles[qc] = osbp.tile([128, 512], BF16, name=f"o{qc}")
                return o_tiles[qc]

            def norm_step(ops_j, qc, j, step):
                key = (qc, j)
                if step == 0:
                    rs = ntmp.tile([1, 512], F32, name=f"rs{j}")
                    nc.vector.tensor_copy(rs[:], ops_j[DH:DH + 1, :])
                    un = ntmp.tile([64, 512], F32, name=f"un{j}")
                    nc.vector.tensor_copy(un[:], ops_j[0:DH, :])
                    norm_state[key] = (un, rs)
                elif step == 1:
                    un, rs = norm_state[key]
                    rr = ntmp.tile([1, 512], F32, name=f"rr{j}")
                    nc.vector.reciprocal_approx_fast(out=rr[:], in_=rs[:])
                    rb = ntmp.tile([64, 512], F32, name=f"rb{j}")
                    nc.gpsimd.partition_broadcast(rb[:], rr[:])
                    norm_state[key] = (un, rb)
                else:
                    un, rb = norm_state[key]
                    o = get_o(qc)
                    nc.gpsimd.tensor_mul(
                        out=o[j * DH:(j + 1) * DH, :], in0=un[:], in1=rb[:])

            def outproj(qc, queue):
                # one matmul per output-channel half: contraction is the full
                # 128 hc of this core's 2 heads (o stacked by partition)
                o = o_tiles[qc]
                for mt in range(2):
                    fps = proj_ps([128, 512])
                    nc.tensor.matmul(fps[:], lhsT=wo_sb[:, mt * 128:(mt + 1) * 128],
                                     rhs=o[:], start=True, stop=True)
                    fo = foutp.tile([128, 512], BF16, name="fo")
                    if mt == 0:
                        nc.scalar.copy(fo[:], fps[:])
                    else:
                        nc.vector.tensor_copy(fo[:], fps[:])
                    queue.dma_start(
                        out=out_d[mt * 128:(mt + 1) * 128,
                                  qc * 512:(qc + 1) * 512],
                        in_=fo[:])

            # ---- attention rounds ----
            def round_(r):
                qc = r
                ops = [outps.tile([128, 512], F32, name=f"ops{j}")
                       for j in range(2)]
                S = SPLIT[r]
                pending = []

                def emit_out(kt, eb):
                    for j in range(2):
                        nc.tensor.matmul(
                            ops[j][:],
                            lhsT=vtt[kt][:, j, :],
                            rhs=eb[:, j * 512:(j + 1) * 512],
                            start=(kt == 0), stop=(kt == NKT - 1))

                for kt2 in range(0, NKT, 2):
                    for kt in (kt2, kt2 + 1):
                        for fn, a in sched_pre[r].get(kt, []):
                            fn(*a)
                        scp = scps.tile([128, 1024], F32, name="scp")
                        for j in range(2):
                            nc.tensor.matmul(
                                scp[:, j * 512:(j + 1) * 512],
                                lhsT=kch[kt // 4][
                                    j * 64:(j + 1) * 64,
                                    (kt % 4) * 128:(kt % 4 + 1) * 128],
                                rhs=qch[qc][j * 64:(j + 1) * 64, :],
                                start=True, stop=True)
                        for fn, a in sched[r].get(kt, []):
                            fn(*a)
                        eb = expb.tile([128, 1024], BF16, name="eb")
                        if S > 0:
                            nc.scalar.activation(
                                eb[:, 0:S], scp[:, 0:S],
                                mybir.ActivationFunctionType.Exp)
                        if S < 1024:
                            nc.vector.tensor_scalar(
                                eb[:, S:1024].bitcast(I16), scp[:, S:1024],
                                SCH_A, SCH_B,
                                mybir.AluOpType.mult, mybir.AluOpType.add)
                        pending.append((kt, eb))
                    while len(pending) > 4:
                        emit_out(*pending.pop(0))
                for it in pending:
                    emit_out(*it)
                return ops

            # ---- pre-round projections ----
            kproj(0)
            qproj(0)

            for r in range(4):
                ops = round_(r)
                items = []
                for j in range(2):
                    items += [
                        (1 + 2 * j, lambda o=ops[j], q=r, h=j: norm_step(o, q, h, 0)),
                        (5 + 2 * j, lambda q=r, h=j: norm_step(None, q, h, 1)),
                        (9 + 2 * j, lambda q=r, h=j: norm_step(None, q, h, 2)),
                    ]
                # sync is idle mid-kernel; keep output DMA issue off the
                # exp-saturated ACT engine entirely
                qdma = [nc.sync, nc.gpsimd, nc.sync][r % 3]
                items += [(14, lambda q=r, qd=qdma: outproj(q, qd))]
                if r < 3:
                    for kt, fn in items:
                        sched[r + 1].setdefault(kt, []).append((fn, ()))
                else:
                    # ---- tail: minimal-latency normalize for the last round.
                    # rowsum -> recip -> PE broadcast-matmul (rb[j*64:...] =
                    # rr[j] for both heads in one N=512 matmul on the
                    # otherwise-idle PE) -> multiplies -> outproj -> dual-queue
                    # output DMA.
                    for j in range(2):
                        nc.scalar.copy(rrin[32 * j:32 * j + 1, :],
                                       ops[j][DH:DH + 1, :])
                    uns = []
                    for j in range(2):
                        un = ntmp.tile([64, 512], F32, name=f"tun{j}")
                        nc.vector.tensor_copy(un[:], ops[j][0:DH, :])
                        uns.append(un)
                    nc.vector.reciprocal_approx_fast(out=rrf[:], in_=rrin[:])
                    nc.vector.tensor_copy(rrb[:], rrf[:])
                    rbps = proj_ps([128, 512])
                    nc.tensor.matmul(rbps[:], lhsT=ind[:], rhs=rrb[:],
                                     start=True, stop=True)
                    o = get_o(3)
                    for j in range(2):
                        nc.vector.tensor_mul(
                            out=o[j * DH:(j + 1) * DH, :], in0=uns[j][:],
                            in1=rbps[j * DH:(j + 1) * DH, :])
                    for mt in range(2):
                        fps = proj_ps([128, 512])
                        nc.tensor.matmul(
                            fps[:], lhsT=wo_sb[:, mt * 128:(mt + 1) * 128],
                            rhs=o[:], start=True, stop=True)
                        fo = foutp.tile([128, 512], BF16, name="fo")
                        if mt == 0:
                            nc.scalar.copy(fo[:], fps[:])
                        else:
                            nc.vector.tensor_copy(fo[:], fps[:])
                        qd = nc.sync if mt == 0 else nc.gpsimd
                        qd.dma_start(
                            out=out_d[mt * 128:(mt + 1) * 128,
                                      3 * 512:4 * 512],
                            in_=fo[:])

    nc.compile()
    return nc


_NC = None


def _get_nc():
    global _NC
    if _NC is None:
        _NC = _build()
    return _NC


def kernel(x, w_qkv, w_out, b_out):
    """Full inputs -> full output, distributed over 8 NeuronCores."""
    _install_ntff_hook()
    nc = _get_nc()

    x = np.asarray(x, dtype=np.float32)
    w_qkv = np.asarray(w_qkv, dtype=np.float32)
    w_out = np.asarray(w_out, dtype=np.float32)
    b_out = np.asarray(b_out, dtype=np.float32)

    bf = ml_dtypes.bfloat16
    xf = x.reshape(B, CH, N)
    # fold the softmax scale into w_q (in fp32, before the bf16 cast)
    wq_t = np.ascontiguousarray((w_qkv[0:HID] * SCALE).T)       # [ch, hid]
    wk_t = np.ascontiguousarray(w_qkv[HID:2 * HID].T)
    wv_t = np.ascontiguousarray(w_qkv[2 * HID:3 * HID].T)
    wo_t = np.ascontiguousarray(w_out.T)                        # [hc, oc]

    wpacks = []
    for hp in range(2):
        s = slice(hp * 128, (hp + 1) * 128)
        wp = np.concatenate(
            [wk_t[0:128, s], wk_t[128:256, s],
             wq_t[0:128, s], wq_t[128:256, s],
             wv_t[0:128, s], wv_t[128:256, s],
             wo_t[s, :]], axis=1)
        wpacks.append(np.ascontiguousarray(wp).astype(bf))

    in_maps = []
    for cid in range(N_CORES):
        b, hp, qh = cid // 4, (cid % 4) // 2, cid % 2
        # rotate the position chunks so chunks 0-1 are this core's q-half
        # (softmax over k positions is permutation-invariant)
        perm = [2 * qh, 2 * qh + 1] + [i for i in range(4)
                                       if i not in (2 * qh, 2 * qh + 1)]
        xb = np.ascontiguousarray(
            xf[b].reshape(CH, 4, 1024)[:, perm, :].reshape(CH, N)).astype(bf)
        in_maps.append({"x": xb, "wp": wpacks[hp]})

    trace = os.environ.get("BASS_KERNEL_TRACE", "0") == "1"
    res = run_bass_kernel_spmd(nc, in_maps, core_ids=list(range(N_CORES)),
                               trace=trace)
    if trace:
        kernel.last_exec_time_ns = res.exec_time_ns

    out = np.zeros((B, CH, N), dtype=np.float32)
    for cid in range(N_CORES):
        b, hp, qh = cid // 4, (cid % 4) // 2, cid % 2
        out[b][:, qh * NQ:(qh + 1) * NQ] += res.results[cid]["out"].astype(
            np.float32)
    out += b_out[None, :, None]
    return out.reshape(B, CH, 64, 64)


kernel.last_exec_time_ns = None


# revision 10
# speedup vs baseline: 1.3582x; 1.3582x over previous
"""Trainium2 Bass kernel for spatial multi-head self-attention (dense_transformer).

Module: x[2,256,64,64] -> qkv 1x1 conv -> 4-head attention over n=4096 spatial
positions -> out 1x1 conv + bias.

Sharding (8 cores): core = (batch b, head-pair hp, query-half qh of 2048
positions). Each core computes K/V for ITS 2 heads over the full 4096
positions (half the projection duplication of a batch/q-quarter split),
Q for its 2 heads over its 2048 positions, the full attention + softmax
for its (batch, head-pair, q-half), and the partial output projection
through its heads' w_out rows. The host sums the two head-pair partials
per (batch, q-half) and adds the bias - no device collectives.

Per-core structure, streaming over 32 k-tiles of 128 positions per
q-chunk round (qc of 512, 4 rounds):
  PE : scoresT[k,q] = k_tile.T @ q (the 2 heads row-packed at partitions
       0:64 / 64:128; the two half-row matmuls run CONCURRENTLY via the
       PE's row-group tiling)
  ACT: exp(scores) PSUM->SBUF bf16 (max-subtraction skipped; scores ~N(0,1)
       by construction so exp cannot overflow). A slice of each tile's
       columns is offloaded to the DVE via a Schraudolph bf16 bit-trick exp.
  PE : out += vT_aug.T @ exp_chunk; vT_aug carries a ones column so row 64
       accumulates the softmax denominator for free (stationary padded to
       128 columns for fast weight load; the extra rows are never read).
  DVE/GpSimd: normalize with reciprocal_approx_fast + partition_broadcast
       + gpsimd multiply (SBUF-only, keeps the DVE free for exp).
K-tiles are walked in PAIRS (scores pairs back-to-back, then the attn@V
matmuls chain weight loads through the PE background weight buffer).
PSUM: three rotating 2-bank score slots (shared with projection groups) +
two 1-bank attention accumulators. Projections stream just-in-time inside
round 0 (kproj/vtproj) with the input DMA pieces ordered by need-time;
normalize and the output projection are deferred into the following
round's schedule. Output is written per-qc as bf16 (host casts/sums in
fp32), spread across DMA queues.

Tail: the last round's denominators go through an ACT rowsum evict ->
DVE recip -> PE broadcast-matmul (ones-block stationary [2,128] spreads
rr[2,512] to rb[128,512] in PSUM, replacing two serialized 1us gpsimd
partition_broadcasts) -> DVE multiplies -> 2 outproj matmuls -> dual-queue
bf16 output DMA.

Exp engine split (EXP_SPLIT, tuned on HW): ACT takes ~60-70% of exp
columns, DVE the rest; round 0 gives the DVE less because it also
carries the kproj/vtproj projection evictions.
"""

import os
import sys
import types

import numpy as np

sys.path.insert(0, "/opt/trn_rl_repo")

import ml_dtypes  # noqa: E402

import concourse.bass as bass  # noqa: E402
import concourse.mybir as mybir  # noqa: E402
import concourse.tile as tile  # noqa: E402
from concourse import bacc  # noqa: E402
from concourse.bass_utils import run_bass_kernel_spmd  # noqa: E402

BF16 = mybir.dt.bfloat16
F32 = mybir.dt.float32
I16 = mybir.dt.int16

N_CORES = 8
CH = 256          # x channels
HID = 256         # qkv hidden (4 heads x 64)
H = 4             # heads
DH = 64           # dim per head
N = 4096          # spatial positions (64*64)
NQ = 2048         # query positions per core (q-half)
B = 2             # batch
SCALE = DH ** -0.5
NKT = N // 128    # 32 k-tiles
NQC = NQ // 512   # 4 q-chunks -> 4 rounds

# Schraudolph exp offload. SPLIT[r]: flat column split point S of the
# per-k-tile score block [128, 2*512] (head-major). The ACT exps columns
# [0:S] exactly; the DVE computes [S:1024] with a one-instruction
# Schraudolph bf16 bit-trick exp (rms rel err ~2% on those columns, largely
# cancelled by the shared softmax denominator).
_SP = os.environ.get("EXP_SPLIT", "704,608,576,576").split(",")
SPLIT = {r: int(_SP[r]) for r in range(4)}
LOG2E = float(np.log2(np.e))
SCH_A = 128.0 * LOG2E
SCH_B = 128.0 * (127.0 - 0.043677)


def _install_ntff_hook():
    """The image's antenv lacks axon_hooks; install it so trace=True works."""
    if "antenv.axon_hooks" in sys.modules:
        return
    try:
        mod = types.ModuleType("antenv.axon_hooks")
        mod._hook = None
        mod.set_axon_ntff_profile_hook = lambda h: setattr(mod, "_hook", h)
        mod.get_axon_ntff_profile_hook = lambda: mod._hook
        sys.modules["antenv.axon_hooks"] = mod
        import antenv
        antenv.axon_hooks = mod
        sys.path.insert(0, "/root/.axon_site/trn_agent_boot")
        from trn_boot import _ntff_profile_via_ctypes
        mod.set_axon_ntff_profile_hook(
            _ntff_profile_via_ctypes("/opt/axon/libaxon_pjrt.so")
        )
    except Exception:
        pass


def _build():
    nc = bacc.Bacc("TRN2", target_bir_lowering=False, debug=False,
                   num_devices=N_CORES)

    x_d = nc.dram_tensor("x", [CH, N], BF16, kind="ExternalInput").ap()
    # per-core weight pack [wk0 wk1 wq0 wq1 wv0 wv1 | wo]: this core's
    # head-pair slices only (128 hid columns), 1024 cols total
    wp_d = nc.dram_tensor("wp", [128, 1024], BF16, kind="ExternalInput").ap()
    out_d = nc.dram_tensor("out", [CH, NQ], BF16, kind="ExternalOutput").ap()

    with tile.TileContext(nc) as tc:
        with tc.tile_pool(name="const", bufs=1) as cst, \
             tc.tile_pool(name="scps", bufs=3, space="PSUM") as scps, \
             tc.tile_pool(name="outps", bufs=1, space="PSUM") as outps, \
             tc.tile_pool(name="expb", bufs=9) as expb, \
             tc.tile_pool(name="osb", bufs=2) as osbp, \
             tc.tile_pool(name="ntmp", bufs=2) as ntmp, \
             tc.tile_pool(name="fout", bufs=2) as foutp:

            def proj_ps(shape):
                return scps.tile(shape, F32, name="scp")

            # ---- persistent tensors ----
            wp_sb = cst.tile([128, 1024], BF16, name="wp")
            wk_sb = [wp_sb[:, c * 128:(c + 1) * 128] for c in range(2)]
            wq_sb = [wp_sb[:, 256 + c * 128:256 + (c + 1) * 128] for c in range(2)]
            wv_sb = [wp_sb[:, 512 + c * 128:512 + (c + 1) * 128] for c in range(2)]
            wo_sb = wp_sb[:, 768:1024]          # [128 hc, 256 oc]
            xb0h = [cst.tile([128, 512], BF16, name=f"xb0h{c}") for c in range(2)]
            xbch = [[cst.tile([128, 1024], BF16, name=f"xb{c}_{i}")
                     for i in range(4)] for c in range(2)]
            kch = [cst.tile([128, 512], BF16, name=f"k{n}") for n in range(8)]
            qch = [cst.tile([128, 512], BF16, name=f"q{qc}") for qc in range(NQC)]
            vtt = [cst.tile([128, 2, 128], BF16, name=f"vt{t}")
                   for t in range(NKT)]
            # tail broadcast stationary: head j's recip row lives at partition
            # 32*j (engine partition bases must be 32-aligned); ind maps row 0
            # -> rb cols 0:64, row 32 -> cols 64:128, other rows are zero so
            # the (memset-to-1.0) filler rows of rr contribute nothing
            ind = cst.tile([33, 128], BF16, name="ind")
            rrin = cst.tile([33, 512], F32, name="rrin")
            rrf = cst.tile([33, 512], F32, name="rrf")
            rrb = cst.tile([33, 512], BF16, name="rrb")

            # ---- input DMAs, need-ordered across the three queues ----
            # sync: critical weights first, then x chunk 2, then wo + x31
            nc.sync.dma_start(out=wp_sb[:, 0:512], in_=wp_d[:, 0:512])
            nc.sync.dma_start(out=wp_sb[:, 512:768], in_=wp_d[:, 512:768])
            nc.sync.dma_start(out=xbch[0][2][:], in_=x_d[0:128, 2048:3072])
            nc.sync.dma_start(out=xbch[1][2][:], in_=x_d[128:256, 2048:3072])
            nc.sync.dma_start(out=wp_sb[:, 768:1024], in_=wp_d[:, 768:1024])
            nc.sync.dma_start(out=xbch[1][3][:], in_=x_d[128:256, 3072:4096])
            # gpsimd: ind/rrin memsets first (feed the ACT table-load dummy)
            nc.gpsimd.memset(ind[:], 0.0)
            nc.gpsimd.memset(ind[0:1, 0:64], 1.0)
            nc.gpsimd.memset(ind[32:33, 64:128], 1.0)
            nc.gpsimd.memset(rrin[:], 1.0)
            nc.gpsimd.dma_start(out=xb0h[0][:], in_=x_d[0:128, 0:512])
            nc.gpsimd.dma_start(out=xbch[0][0][:], in_=x_d[0:128, 0:1024])
            nc.gpsimd.dma_start(out=xbch[0][1][:], in_=x_d[0:128, 1024:2048])
            nc.gpsimd.dma_start(out=xbch[0][3][:], in_=x_d[0:128, 3072:4096])
            # scalar: first xb0h piece, then pre-pull the exp ACT table with a
            # dummy activation (so the ~1.3us table load is off the critical
            # path of round 0's first exp), then the rest of its x chunks
            nc.scalar.dma_start(out=xb0h[1][:], in_=x_d[128:256, 0:512])
            nc.scalar.dma_start(out=xbch[1][0][:], in_=x_d[128:256, 0:1024])
            nc.scalar.activation(rrin[0:1, 0:1], ind[0:1, 0:1],
                                 mybir.ActivationFunctionType.Exp)
            nc.scalar.dma_start(out=xbch[1][1][:], in_=x_d[128:256, 1024:2048])

            # ---- projection emitters ----
            def kproj(n):
                ps = proj_ps([128, 512])
                for c in range(2):
                    rhs = (xb0h[c][:] if n == 0 else
                           xbch[c][n // 2][:, (n % 2) * 512:(n % 2 + 1) * 512])
                    nc.tensor.matmul(ps[:], lhsT=wk_sb[c][:], rhs=rhs,
                                     start=(c == 0), stop=(c == 1))
                nc.vector.tensor_copy(kch[n][:], ps[:])

            def qproj(qc):
                ps = proj_ps([128, 512])
                for c in range(2):
                    rhs = (xb0h[c][:] if qc == 0 else
                           xbch[c][qc // 2][:, (qc % 2) * 512:(qc % 2 + 1) * 512])
                    nc.tensor.matmul(ps[:], lhsT=wq_sb[c][:], rhs=rhs,
                                     start=(c == 0), stop=(c == 1))
                # qc0's eviction rides the idle ACT so it doesn't queue behind
                # kch[0]'s eviction on the DVE (both gate scores kt=0)
                if qc == 0:
                    nc.scalar.copy(qch[qc][:], ps[:])
                else:
                    nc.vector.tensor_copy(qch[qc][:], ps[:])

            def vtproj2(tp):
                ps = proj_ps([128, 256])
                for u in range(2):
                    t = 2 * tp + u
                    for c in range(2):
                        nc.tensor.matmul(
                            ps[:, u * 128:(u + 1) * 128],
                            lhsT=xbch[c][t // 8][:, (t % 8) * 128:(t % 8 + 1) * 128],
                            rhs=wv_sb[c],
                            start=(c == 0), stop=(c == 1))
                for u in range(2):
                    t = 2 * tp + u
                    nc.gpsimd.memset(vtt[t][:, :, DH:DH + 1], 1.0)
                    if u == 0:
                        nc.scalar.copy(
                            vtt[t][:, :, 0:DH],
                            ps[:, u * 128:(u + 1) * 128].rearrange(
                                "p (h d) -> p h d", d=DH))
                    else:
                        nc.vector.tensor_copy(
                            vtt[t][:, :, 0:DH],
                            ps[:, u * 128:(u + 1) * 128].rearrange(
                                "p (h d) -> p h d", d=DH))

            # ---- interleave schedules: round index -> {kt: [thunks]} ----
            # Round r == q-chunk r. Round 0 carries kproj(1..7) + vtproj JIT
            # (kproj(n) feeds scores kt=4n; vtproj2(tp) feeds attn@V kt=2tp,
            # which fires ~5 k-tiles behind scores). qproj(qc) spread out.
            sched = {r: {} for r in range(4)}
            sched_pre = {r: {} for r in range(4)}

            def add(r, kt, fn, *a):
                sched[r].setdefault(kt, []).append((fn, a))

            for n in range(1, 8):
                add(0, max(2, 4 * n - 3), kproj, n)
            for tp in range(NKT // 2):
                kt = 2 * tp + 2 if tp < 14 else (29 if tp == 14 else 30)
                add(0, kt, vtproj2, tp)
            add(0, 12, qproj, 1)
            add(1, 4, qproj, 2)
            add(2, 4, qproj, 3)

            # ---- deferred finishers (normalize / out-projection) ----
            o_tiles = {}   # qc -> [128, 512] bf16 (both heads stacked)
            norm_state = {}

            def get_o(qc):
                if qc not in o_tiles:
                    o_tiles[qc] = osbp.tile([128, 512], BF16, name=f"o{qc}")
                return o_tiles[qc]

            def norm_step(ops_j, qc, j, step):
                key = (qc, j)
                if step == 0:
                    rs = ntmp.tile([1, 512], F32, name=f"rs{j}")
                    nc.vector.tensor_copy(rs[:], ops_j[DH:DH + 1, :])
                    un = ntmp.tile([64, 512], F32, name=f"un{j}")
                    nc.vector.tensor_copy(un[:], ops_j[0:DH, :])
                    norm_state[key] = (un, rs)
                elif step == 1:
                    un, rs = norm_state[key]
                    rr = ntmp.tile([1, 512], F32, name=f"rr{j}")
                    nc.vector.reciprocal_approx_fast(out=rr[:], in_=rs[:])
                    rb = ntmp.tile([64, 512], F32, name=f"rb{j}")
                    nc.gpsimd.partition_broadcast(rb[:], rr[:])
                    norm_state[key] = (un, rb)
                else:
                    # DVE, not gpsimd: a gpsimd tensor op would swap the
                    # ext-isa IRAM away from partition_broadcast (~6.7us
                    # reload each way, measured)
                    un, rb = norm_state[key]
                    o = get_o(qc)
                    nc.vector.tensor_mul(
                        out=o[j * DH:(j + 1) * DH, :], in0=un[:], in1=rb[:])

            def outproj(qc, queue):
                # one matmul per output-channel half: contraction is the full
                # 128 hc of this core's 2 heads (o stacked by partition)
                o = o_tiles[qc]
                for mt in range(2):
                    fps = proj_ps([128, 512])
                    nc.tensor.matmul(fps[:], lhsT=wo_sb[:, mt * 128:(mt + 1) * 128],
                                     rhs=o[:], start=True, stop=True)
                    fo = foutp.tile([128, 512], BF16, name="fo")
                    if mt == 0:
                        nc.scalar.copy(fo[:], fps[:])
                    else:
                        nc.vector.tensor_copy(fo[:], fps[:])
                    queue.dma_start(
                        out=out_d[mt * 128:(mt + 1) * 128,
                                  qc * 512:(qc + 1) * 512],
                        in_=fo[:])

            # ---- attention rounds ----
            def round_(r):
                qc = r
                ops = [outps.tile([128, 512], F32, name=f"ops{j}")
                       for j in range(2)]
                S = SPLIT[r]
                pending = []

                def emit_out(kt, eb):
                    for j in range(2):
                        nc.tensor.matmul(
                            ops[j][:],
                            lhsT=vtt[kt][:, j, :],
                            rhs=eb[:, j * 512:(j + 1) * 512],
                            start=(kt == 0), stop=(kt == NKT - 1))

                for kt2 in range(0, NKT, 2):
                    for kt in (kt2, kt2 + 1):
                        for fn, a in sched_pre[r].get(kt, []):
                            fn(*a)
                        scp = scps.tile([128, 1024], F32, name="scp")
                        for j in range(2):
                            nc.tensor.matmul(
                                scp[:, j * 512:(j + 1) * 512],
                                lhsT=kch[kt // 4][
                                    j * 64:(j + 1) * 64,
                                    (kt % 4) * 128:(kt % 4 + 1) * 128],
                                rhs=qch[qc][j * 64:(j + 1) * 64, :],
                                start=True, stop=True)
                        for fn, a in sched[r].get(kt, []):
                            fn(*a)
                        eb = expb.tile([128, 1024], BF16, name="eb")
                        if S > 0:
                            nc.scalar.activation(
                                eb[:, 0:S], scp[:, 0:S],
                                mybir.ActivationFunctionType.Exp)
                        if S < 1024:
                            nc.vector.tensor_scalar(
                                eb[:, S:1024].bitcast(I16), scp[:, S:1024],
                                SCH_A, SCH_B,
                                mybir.AluOpType.mult, mybir.AluOpType.add)
                        pending.append((kt, eb))
                    while len(pending) > 4:
                        emit_out(*pending.pop(0))
                for it in pending:
                    emit_out(*it)
                return ops

            # ---- pre-round projections ----
            kproj(0)
            qproj(0)

            for r in range(4):
                ops = round_(r)
                items = []
                for j in range(2):
                    items += [
                        (1 + 2 * j, lambda o=ops[j], q=r, h=j: norm_step(o, q, h, 0)),
                        (5 + 2 * j, lambda q=r, h=j: norm_step(None, q, h, 1)),
                        (9 + 2 * j, lambda q=r, h=j: norm_step(None, q, h, 2)),
                    ]
                # sync is idle mid-kernel; keep output DMA issue off the
                # exp-saturated ACT engine entirely
                qdma = [nc.sync, nc.gpsimd, nc.sync][r % 3]
                items += [(14, lambda q=r, qd=qdma: outproj(q, qd))]
                if r < 3:
                    for kt, fn in items:
                        sched[r + 1].setdefault(kt, []).append((fn, ()))
                else:
                    # ---- tail: minimal-latency normalize for the last round.
                    # rowsum -> recip -> PE broadcast-matmul (rb[j*64:...] =
                    # rr[j] for both heads in one N=512 matmul on the
                    # otherwise-idle PE) -> multiplies -> outproj -> dual-queue
                    # output DMA.
                    for j in range(2):
                        nc.scalar.copy(rrin[32 * j:32 * j + 1, :],
                                       ops[j][DH:DH + 1, :])
                    uns = []
                    for j in range(2):
                        un = ntmp.tile([64, 512], F32, name=f"tun{j}")
                        nc.vector.tensor_copy(un[:], ops[j][0:DH, :])
                        uns.append(un)
                    nc.vector.reciprocal_approx_fast(out=rrf[:], in_=rrin[:])
                    nc.vector.tensor_copy(rrb[:], rrf[:])
                    rbps = proj_ps([128, 512])
                    nc.tensor.matmul(rbps[:], lhsT=ind[:], rhs=rrb[:],
                                     start=True, stop=True)
                    o = get_o(3)
                    for j in range(2):
                        nc.vector.tensor_mul(
                            out=o[j * DH:(j + 1) * DH, :], in0=uns[j][:],
                            in1=rbps[j * DH:(j + 1) * DH, :])
                    for mt in range(2):
                        fps = proj_ps([128, 512])
                        nc.tensor.matmul(
                            fps[:], lhsT=wo_sb[:, mt * 128:(mt + 1) * 128],
                            rhs=o[:], start=True, stop=True)
                        fo = foutp.tile([128, 512], BF16, name="fo")
                        if mt == 0:
                            nc.scalar.copy(fo[:], fps[:])
                        else:
                            nc.vector.tensor_copy(fo[:], fps[:])
                        qd = nc.sync if mt == 0 else nc.gpsimd
                        qd.dma_start(
                            out=out_d[mt * 128:(mt + 1) * 128,
                                      3 * 512:4 * 512],
                            in_=fo[:])

    nc.compile()
    return nc


_NC = None


def _get_nc():
    global _NC
    if _NC is None:
        _NC = _build()
    return _NC


def kernel(x, w_qkv, w_out, b_out):
    """Full inputs -> full output, distributed over 8 NeuronCores."""
    _install_ntff_hook()
    nc = _get_nc()

    x = np.asarray(x, dtype=np.float32)
    w_qkv = np.asarray(w_qkv, dtype=np.float32)
    w_out = np.asarray(w_out, dtype=np.float32)
    b_out = np.asarray(b_out, dtype=np.float32)

    bf = ml_dtypes.bfloat16
    xf = x.reshape(B, CH, N)
    # fold the softmax scale into w_q (in fp32, before the bf16 cast)
    wq_t = np.ascontiguousarray((w_qkv[0:HID] * SCALE).T)       # [ch, hid]
    wk_t = np.ascontiguousarray(w_qkv[HID:2 * HID].T)
    wv_t = np.ascontiguousarray(w_qkv[2 * HID:3 * HID].T)
    wo_t = np.ascontiguousarray(w_out.T)                        # [hc, oc]

    wpacks = []
    for hp in range(2):
        s = slice(hp * 128, (hp + 1) * 128)
        wp = np.concatenate(
            [wk_t[0:128, s], wk_t[128:256, s],
             wq_t[0:128, s], wq_t[128:256, s],
             wv_t[0:128, s], wv_t[128:256, s],
             wo_t[s, :]], axis=1)
        wpacks.append(np.ascontiguousarray(wp).astype(bf))

    in_maps = []
    for cid in range(N_CORES):
        b, hp, qh = cid // 4, (cid % 4) // 2, cid % 2
        # rotate the position chunks so chunks 0-1 are this core's q-half
        # (softmax over k positions is permutation-invariant)
        perm = [2 * qh, 2 * qh + 1] + [i for i in range(4)
                                       if i not in (2 * qh, 2 * qh + 1)]
        xb = np.ascontiguousarray(
            xf[b].reshape(CH, 4, 1024)[:, perm, :].reshape(CH, N)).astype(bf)
        in_maps.append({"x": xb, "wp": wpacks[hp]})

    trace = os.environ.get("BASS_KERNEL_TRACE", "0") == "1"
    res = run_bass_kernel_spmd(nc, in_maps, core_ids=list(range(N_CORES)),
                               trace=trace)
    if trace:
        kernel.last_exec_time_ns = res.exec_time_ns

    out = np.zeros((B, CH, N), dtype=np.float32)
    for cid in range(N_CORES):
        b, hp, qh = cid // 4, (cid % 4) // 2, cid % 2
        out[b][:, qh * NQ:(qh + 1) * NQ] += res.results[cid]["out"].astype(
            np.float32)
    out += b_out[None, :, None]
    return out.reshape(B, CH, 64, 64)


kernel.last_exec_time_ns = None


# revision 16
# speedup vs baseline: 1.3680x; 1.0072x over previous
"""Trainium2 Bass kernel for spatial multi-head self-attention (dense_transformer).

Module: x[2,256,64,64] -> qkv 1x1 conv -> 4-head attention over n=4096 spatial
positions -> out 1x1 conv + bias.

Sharding (8 cores): core = (batch b, head-pair hp, query-half qh of 2048
positions). Each core computes K/V for ITS 2 heads over the full 4096
positions (half the projection duplication of a batch/q-quarter split),
Q for its 2 heads over its 2048 positions, the full attention + softmax
for its (batch, head-pair, q-half), and the partial output projection
through its heads' w_out rows. The host sums the two head-pair partials
per (batch, q-half) and adds the bias - no device collectives.

Per-core structure, streaming over 32 k-tiles of 128 positions per
q-chunk round (qc of 512, 4 rounds):
  PE : scoresT[k,q] = k_tile.T @ q (the 2 heads row-packed at partitions
       0:64 / 64:128; the two half-row matmuls run CONCURRENTLY via the
       PE's row-group tiling)
  ACT: exp(scores) PSUM->SBUF bf16 (max-subtraction skipped; scores ~N(0,1)
       by construction so exp cannot overflow). A slice of each tile's
       columns is offloaded to the DVE via a Schraudolph bf16 bit-trick exp.
  PE : out += vT_aug.T @ exp_chunk; vT_aug carries a ones column so row 64
       accumulates the softmax denominator for free (stationary padded to
       128 columns for fast weight load; the extra rows are never read).
  DVE/GpSimd: normalize with reciprocal_approx_fast + partition_broadcast
       + gpsimd multiply (SBUF-only, keeps the DVE free for exp).
K-tiles are walked in PAIRS (scores pairs back-to-back, then the attn@V
matmuls chain weight loads through the PE background weight buffer).
PSUM: three rotating 2-bank score slots (shared with projection groups) +
two 1-bank attention accumulators. Projections stream just-in-time inside
round 0 (kproj/vtproj) with the input DMA pieces ordered by need-time;
normalize and the output projection are deferred into the following
round's schedule. Output is written per-qc as bf16 (host casts/sums in
fp32), spread across DMA queues.

Tail: the last round's denominators go through an ACT rowsum evict ->
DVE recip -> PE broadcast-matmul (ones-block stationary [2,128] spreads
rr[2,512] to rb[128,512] in PSUM, replacing two serialized 1us gpsimd
partition_broadcasts) -> DVE multiplies -> 2 outproj matmuls -> dual-queue
bf16 output DMA.

Exp engine split (EXP_SPLIT, tuned on HW): ACT takes ~60-70% of exp
columns, DVE the rest; round 0 gives the DVE less because it also
carries the kproj/vtproj projection evictions.
"""

import os
import sys
import types

import numpy as np

sys.path.insert(0, "/opt/trn_rl_repo")

import ml_dtypes  # noqa: E402

import concourse.bass as bass  # noqa: E402
import concourse.mybir as mybir  # noqa: E402
import concourse.tile as tile  # noqa: E402
from concourse import bacc  # noqa: E402
from concourse.bass_utils import run_bass_kernel_spmd  # noqa: E402

BF16 = mybir.dt.bfloat16
F32 = mybir.dt.float32
I16 = mybir.dt.int16

N_CORES = 8
CH = 256          # x channels
HID = 256         # qkv hidden (4 heads x 64)
H = 4             # heads
DH = 64           # dim per head
N = 4096          # spatial positions (64*64)
NQ = 2048         # query positions per core (q-half)
B = 2             # batch
SCALE = DH ** -0.5
NKT = N // 128    # 32 k-tiles
NQC = NQ // 512   # 4 q-chunks -> 4 rounds

# Schraudolph exp offload. SPLIT[r]: flat column split point S of the
# per-k-tile score block [128, 2*512] (head-major). The ACT exps columns
# [0:S] exactly; the DVE computes [S:1024] with a one-instruction
# Schraudolph bf16 bit-trick exp (rms rel err ~2% on those columns, largely
# cancelled by the shared softmax denominator).
_SP = os.environ.get("EXP_SPLIT", "704,640,640,640").split(",")
SPLIT = {r: int(_SP[r]) for r in range(4)}
LOG2E = float(np.log2(np.e))
SCH_A = 128.0 * LOG2E
SCH_B = 128.0 * (127.0 - 0.043677)


def _install_ntff_hook():
    """The image's antenv lacks axon_hooks; install it so trace=True works."""
    if "antenv.axon_hooks" in sys.modules:
        return
    try:
        mod = types.ModuleType("antenv.axon_hooks")
        mod._hook = None
        mod.set_axon_ntff_profile_hook = lambda h: setattr(mod, "_hook", h)
        mod.get_axon_ntff_profile_hook = lambda: mod._hook
        sys.modules["antenv.axon_hooks"] = mod
        import antenv
        antenv.axon_hooks = mod
        sys.path.insert(0, "/root/.axon_site/trn_agent_boot")
        from trn_boot import _ntff_profile_via_ctypes
        mod.set_axon_ntff_profile_hook(
            _ntff_profile_via_ctypes("/opt/axon/libaxon_pjrt.so")
        )
    except Exception:
        pass


def _build():
    nc = bacc.Bacc("TRN2", target_bir_lowering=False, debug=False,
                   num_devices=N_CORES)

    x_d = nc.dram_tensor("x", [CH, N], BF16, kind="ExternalInput").ap()
    # per-core weight pack [wk0 wk1 wq0 wq1 wv0 wv1 | wo]: this core's
    # head-pair slices only (128 hid columns), 1024 cols total
    wp_d = nc.dram_tensor("wp", [128, 1024], BF16, kind="ExternalInput").ap()
    out_d = nc.dram_tensor("out", [CH, NQ], BF16, kind="ExternalOutput").ap()

    with tile.TileContext(nc) as tc:
        with tc.tile_pool(name="const", bufs=1) as cst, \
             tc.tile_pool(name="scps", bufs=3, space="PSUM") as scps, \
             tc.tile_pool(name="outps", bufs=1, space="PSUM") as outps, \
             tc.tile_pool(name="expb", bufs=9) as expb, \
             tc.tile_pool(name="osb", bufs=2) as osbp, \
             tc.tile_pool(name="ntmp", bufs=2) as ntmp, \
             tc.tile_pool(name="fout", bufs=2) as foutp:

            def proj_ps(shape):
                return scps.tile(shape, F32, name="scp")

            # ---- persistent tensors ----
            wp_sb = cst.tile([128, 1024], BF16, name="wp")
            wk_sb = [wp_sb[:, c * 128:(c + 1) * 128] for c in range(2)]
            wq_sb = [wp_sb[:, 256 + c * 128:256 + (c + 1) * 128] for c in range(2)]
            wv_sb = [wp_sb[:, 512 + c * 128:512 + (c + 1) * 128] for c in range(2)]
            wo_sb = wp_sb[:, 768:1024]          # [128 hc, 256 oc]
            xb0h = [cst.tile([128, 512], BF16, name=f"xb0h{c}") for c in range(2)]
            xbch = [[cst.tile([128, 1024], BF16, name=f"xb{c}_{i}")
                     for i in range(4)] for c in range(2)]
            kch = [cst.tile([128, 512], BF16, name=f"k{n}") for n in range(8)]
            qch = [cst.tile([128, 512], BF16, name=f"q{qc}") for qc in range(NQC)]
            vtt = [cst.tile([128, 2, 128], BF16, name=f"vt{t}")
                   for t in range(NKT)]
            # tail broadcast stationary: head j's recip row lives at partition
            # 32*j (engine partition bases must be 32-aligned); ind maps row 0
            # -> rb cols 0:64, row 32 -> cols 64:128, other rows are zero so
            # the (memset-to-1.0) filler rows of rr contribute nothing
            ind = cst.tile([33, 128], BF16, name="ind")
            rrin = cst.tile([33, 512], F32, name="rrin")
            rrf = cst.tile([33, 512], F32, name="rrf")
            rrb = cst.tile([33, 512], BF16, name="rrb")

            # ---- input DMAs, need-ordered across the three queues ----
            # sync: critical weights first, then x chunk 2, then wo + x31
            nc.sync.dma_start(out=wp_sb[:, 0:512], in_=wp_d[:, 0:512])
            nc.sync.dma_start(out=wp_sb[:, 512:768], in_=wp_d[:, 512:768])
            nc.sync.dma_start(out=xbch[0][2][:], in_=x_d[0:128, 2048:3072])
            nc.sync.dma_start(out=xbch[1][2][:], in_=x_d[128:256, 2048:3072])
            nc.sync.dma_start(out=wp_sb[:, 768:1024], in_=wp_d[:, 768:1024])
            nc.sync.dma_start(out=xbch[1][3][:], in_=x_d[128:256, 3072:4096])
            # gpsimd: tiny ind memsets (feed the warmup matmuls + dummy exp),
            # then its x DMA issues; rrin's big memset rides the idle DVE
            nc.gpsimd.memset(ind[:], 0.0)
            nc.gpsimd.memset(ind[0:1, 0:64], 1.0)
            nc.gpsimd.memset(ind[32:33, 64:128], 1.0)
            nc.vector.memset(rrin[:], 1.0)
            nc.vector.memset(rrb[:], 1.0)
            nc.gpsimd.dma_start(out=xb0h[0][:], in_=x_d[0:128, 0:512])
            nc.gpsimd.dma_start(out=xbch[0][0][:], in_=x_d[0:128, 0:1024])
            nc.gpsimd.dma_start(out=xbch[0][1][:], in_=x_d[0:128, 1024:2048])
            nc.gpsimd.dma_start(out=xbch[0][3][:], in_=x_d[0:128, 3072:4096])
            # scalar: first xb0h piece, then pre-pull the exp ACT table with a
            # dummy activation (so the ~1.3us table load is off the critical
            # path of round 0's first exp), then the rest of its x chunks
            nc.scalar.dma_start(out=xb0h[1][:], in_=x_d[128:256, 0:512])
            nc.scalar.dma_start(out=xbch[1][0][:], in_=x_d[128:256, 0:1024])
            nc.scalar.activation(rrf[0:1, 0:1], ind[0:1, 0:1],
                                 mybir.ActivationFunctionType.Exp)
            nc.scalar.dma_start(out=xbch[1][1][:], in_=x_d[128:256, 1024:2048])
            # PE warmup: a chain of junk matmuls on already-initialized tiles
            # keeps the HAM activity window busy while the input DMA streams,
            # so the first real projections run at 2.4GHz instead of 1.2
            wps = scps.tile([128, 512], F32, name="scp")
            for _ in range(7):
                nc.tensor.matmul(wps[:], lhsT=ind[:, :], rhs=rrb[:, :],
                                 start=True, stop=True)

            # ---- projection emitters ----
            def kproj(n):
                ps = proj_ps([128, 512])
                for c in range(2):
                    rhs = (xb0h[c][:] if n == 0 else
                           xbch[c][n // 2][:, (n % 2) * 512:(n % 2 + 1) * 512])
                    nc.tensor.matmul(ps[:], lhsT=wk_sb[c][:], rhs=rhs,
                                     start=(c == 0), stop=(c == 1))
                nc.vector.tensor_copy(kch[n][:], ps[:])

            def qproj(qc):
                ps = proj_ps([128, 512])
                for c in range(2):
                    rhs = (xb0h[c][:] if qc == 0 else
                           xbch[c][qc // 2][:, (qc % 2) * 512:(qc % 2 + 1) * 512])
                    nc.tensor.matmul(ps[:], lhsT=wq_sb[c][:], rhs=rhs,
                                     start=(c == 0), stop=(c == 1))
                # qc0's eviction rides the idle ACT so it doesn't queue behind
                # kch[0]'s eviction on the DVE (both gate scores kt=0)
                if qc == 0:
                    nc.scalar.copy(qch[qc][:], ps[:])
                else:
                    nc.vector.tensor_copy(qch[qc][:], ps[:])

            def vtproj2(tp):
                ps = proj_ps([128, 256])
                for u in range(2):
                    t = 2 * tp + u
                    for c in range(2):
                        nc.tensor.matmul(
                            ps[:, u * 128:(u + 1) * 128],
                            lhsT=xbch[c][t // 8][:, (t % 8) * 128:(t % 8 + 1) * 128],
                            rhs=wv_sb[c],
                            start=(c == 0), stop=(c == 1))
                for u in range(2):
                    t = 2 * tp + u
                    nc.gpsimd.memset(vtt[t][:, :, DH:DH + 1], 1.0)
                    if u == 0:
                        nc.scalar.copy(
                            vtt[t][:, :, 0:DH],
                            ps[:, u * 128:(u + 1) * 128].rearrange(
                                "p (h d) -> p h d", d=DH))
                    else:
                        nc.vector.tensor_copy(
                            vtt[t][:, :, 0:DH],
                            ps[:, u * 128:(u + 1) * 128].rearrange(
                                "p (h d) -> p h d", d=DH))

            # ---- interleave schedules: round index -> {kt: [thunks]} ----
            # Round r == q-chunk r. Round 0 carries kproj(1..7) + vtproj JIT
            # (kproj(n) feeds scores kt=4n; vtproj2(tp) feeds attn@V kt=2tp,
            # which fires ~5 k-tiles behind scores). qproj(qc) spread out.
            sched = {r: {} for r in range(4)}
            sched_pre = {r: {} for r in range(4)}

            def add(r, kt, fn, *a):
                sched[r].setdefault(kt, []).append((fn, a))

            for n in range(1, 8):
                add(0, max(2, 4 * n - 3), kproj, n)
            for tp in range(NKT // 2):
                kt = 2 * tp + 2 if tp < 14 else (29 if tp == 14 else 30)
                add(0, kt, vtproj2, tp)
            add(0, 12, qproj, 1)
            add(1, 4, qproj, 2)
            add(2, 4, qproj, 3)

            # ---- deferred finishers (normalize / out-projection) ----
            o_tiles = {}   # qc -> [128, 512] bf16 (both heads stacked)
            norm_state = {}

            def get_o(qc):
                if qc not in o_tiles:
                    o_tiles[qc] = osbp.tile([128, 512], BF16, name=f"o{qc}")
                return o_tiles[qc]

            def norm_step(ops_j, qc, j, step):
                key = (qc, j)
                if step == 0:
                    rs = ntmp.tile([1, 512], F32, name=f"rs{j}")
                    nc.vector.tensor_copy(rs[:], ops_j[DH:DH + 1, :])
                    un = ntmp.tile([64, 512], F32, name=f"un{j}")
                    nc.vector.tensor_copy(un[:], ops_j[0:DH, :])
                    norm_state[key] = (un, rs)
                elif step == 1:
                    un, rs = norm_state[key]
                    rr = ntmp.tile([1, 512], F32, name=f"rr{j}")
                    nc.vector.reciprocal_approx_fast(out=rr[:], in_=rs[:])
                    rb = ntmp.tile([64, 512], F32, name=f"rb{j}")
                    nc.gpsimd.partition_broadcast(rb[:], rr[:])
                    norm_state[key] = (un, rb)
                else:
                    # DVE, not gpsimd: a gpsimd tensor op would swap the
                    # ext-isa IRAM away from partition_broadcast (~6.7us
                    # reload each way, measured)
                    un, rb = norm_state[key]
                    o = get_o(qc)
                    nc.vector.tensor_mul(
                        out=o[j * DH:(j + 1) * DH, :], in0=un[:], in1=rb[:])

            def outproj(qc, queue):
                # one matmul per output-channel half: contraction is the full
                # 128 hc of this core's 2 heads (o stacked by partition)
                o = o_tiles[qc]
                for mt in range(2):
                    fps = proj_ps([128, 512])
                    nc.tensor.matmul(fps[:], lhsT=wo_sb[:, mt * 128:(mt + 1) * 128],
                                     rhs=o[:], start=True, stop=True)
                    fo = foutp.tile([128, 512], BF16, name="fo")
                    if mt == 0:
                        nc.scalar.copy(fo[:], fps[:])
                    else:
                        nc.vector.tensor_copy(fo[:], fps[:])
                    queue.dma_start(
                        out=out_d[mt * 128:(mt + 1) * 128,
                                  qc * 512:(qc + 1) * 512],
                        in_=fo[:])

            # ---- attention rounds ----
            def round_(r):
                qc = r
                ops = [outps.tile([128, 512], F32, name=f"ops{j}")
                       for j in range(2)]
                S = SPLIT[r]
                pending = []

                def emit_out(kt, eb):
                    for j in range(2):
                        nc.tensor.matmul(
                            ops[j][:],
                            lhsT=vtt[kt][:, j, :],
                            rhs=eb[:, j * 512:(j + 1) * 512],
                            start=(kt == 0), stop=(kt == NKT - 1))

                for kt2 in range(0, NKT, 2):
                    for kt in (kt2, kt2 + 1):
                        for fn, a in sched_pre[r].get(kt, []):
                            fn(*a)
                        scp = scps.tile([128, 1024], F32, name="scp")
                        for j in range(2):
                            nc.tensor.matmul(
                                scp[:, j * 512:(j + 1) * 512],
                                lhsT=kch[kt // 4][
                                    j * 64:(j + 1) * 64,
                                    (kt % 4) * 128:(kt % 4 + 1) * 128],
                                rhs=qch[qc][j * 64:(j + 1) * 64, :],
                                start=True, stop=True)
                        for fn, a in sched[r].get(kt, []):
                            fn(*a)
                        eb = expb.tile([128, 1024], BF16, name="eb")
                        if S > 0:
                            nc.scalar.activation(
                                eb[:, 0:S], scp[:, 0:S],
                                mybir.ActivationFunctionType.Exp)
                        if S < 1024:
                            nc.vector.tensor_scalar(
                                eb[:, S:1024].bitcast(I16), scp[:, S:1024],
                                SCH_A, SCH_B,
                                mybir.AluOpType.mult, mybir.AluOpType.add)
                        pending.append((kt, eb))
                    while len(pending) > 6:
                        emit_out(*pending.pop(0))
                for it in pending:
                    emit_out(*it)
                return ops

            # ---- pre-round projections ----
            kproj(0)
            qproj(0)

            for r in range(4):
                ops = round_(r)
                items = []
                for j in range(2):
                    items += [
                        (1 + j, lambda o=ops[j], q=r, h=j: norm_step(o, q, h, 0)),
                        (4 + 2 * j, lambda q=r, h=j: norm_step(None, q, h, 1)),
                        (8 + 2 * j, lambda q=r, h=j: norm_step(None, q, h, 2)),
                    ]
                # sync is idle mid-kernel; keep output DMA issue off the
                # exp-saturated ACT engine entirely
                qdma = [nc.sync, nc.gpsimd, nc.sync][r % 3]
                items += [(16, lambda q=r, qd=qdma: outproj(q, qd))]
                if r < 3:
                    for kt, fn in items:
                        sched[r + 1].setdefault(kt, []).append((fn, ()))
                else:
                    # ---- tail: minimal-latency normalize for the last round.
                    # rowsum -> recip -> PE broadcast-matmul (rb[j*64:...] =
                    # rr[j] for both heads in one N=512 matmul on the
                    # otherwise-idle PE) -> multiplies -> outproj -> dual-queue
                    # output DMA.
                    for j in range(2):
                        nc.scalar.copy(rrin[32 * j:32 * j + 1, :],
                                       ops[j][DH:DH + 1, :])
                    uns = []
                    for j in range(2):
                        un = ntmp.tile([64, 512], F32, name=f"tun{j}")
                        nc.vector.tensor_copy(un[:], ops[j][0:DH, :])
                        uns.append(un)
                    nc.vector.reciprocal_approx_fast(out=rrf[:], in_=rrin[:])
                    nc.vector.tensor_copy(rrb[:], rrf[:])
                    rbps = proj_ps([128, 512])
                    nc.tensor.matmul(rbps[:], lhsT=ind[:], rhs=rrb[:],
                                     start=True, stop=True)
                    o = get_o(3)
                    for j in range(2):
                        nc.vector.tensor_mul(
                            out=o[j * DH:(j + 1) * DH, :], in0=uns[j][:],
                            in1=rbps[j * DH:(j + 1) * DH, :])
                    for mt in range(2):
                        fps = proj_ps([128, 512])
                        nc.tensor.matmul(
                            fps[:], lhsT=wo_sb[:, mt * 128:(mt + 1) * 128],
                            rhs=o[:], start=True, stop=True)
                        fo = foutp.tile([128, 512], BF16, name="fo")
                        if mt == 0:
                            nc.scalar.copy(fo[:], fps[:])
                        else:
                            nc.vector.tensor_copy(fo[:], fps[:])
                        qd = nc.sync if mt == 0 else nc.gpsimd
                        qd.dma_start(
                            out=out_d[mt * 128:(mt + 1) * 128,
                                      3 * 512:4 * 512],
                            in_=fo[:])

    nc.compile()
    return nc


_NC = None


def _get_nc():
    global _NC
    if _NC is None:
        _NC = _build()
    return _NC


def kernel(x, w_qkv, w_out, b_out):
    """Full inputs -> full output, distributed over 8 NeuronCores."""
    _install_ntff_hook()
    nc = _get_nc()

    x = np.asarray(x, dtype=np.float32)
    w_qkv = np.asarray(w_qkv, dtype=np.float32)
    w_out = np.asarray(w_out, dtype=np.float32)
    b_out = np.asarray(b_out, dtype=np.float32)

    bf = ml_dtypes.bfloat16
    xf = x.reshape(B, CH, N)
    # fold the softmax scale into w_q (in fp32, before the bf16 cast)
    wq_t = np.ascontiguousarray((w_qkv[0:HID] * SCALE).T)       # [ch, hid]
    wk_t = np.ascontiguousarray(w_qkv[HID:2 * HID].T)
    wv_t = np.ascontiguousarray(w_qkv[2 * HID:3 * HID].T)
    wo_t = np.ascontiguousarray(w_out.T)                        # [hc, oc]

    wpacks = []
    for hp in range(2):
        s = slice(hp * 128, (hp + 1) * 128)
        wp = np.concatenate(
            [wk_t[0:128, s], wk_t[128:256, s],
             wq_t[0:128, s], wq_t[128:256, s],
             wv_t[0:128, s], wv_t[128:256, s],
             wo_t[s, :]], axis=1)
        wpacks.append(np.ascontiguousarray(wp).astype(bf))

    in_maps = []
    for cid in range(N_CORES):
        b, hp, qh = cid // 4, (cid % 4) // 2, cid % 2
        # rotate the position chunks so chunks 0-1 are this core's q-half
        # (softmax over k positions is permutation-invariant)
        perm = [2 * qh, 2 * qh + 1] + [i for i in range(4)
                                       if i not in (2 * qh, 2 * qh + 1)]
        xb = np.ascontiguousarray(
            xf[b].reshape(CH, 4, 1024)[:, perm, :].reshape(CH, N)).astype(bf)
        in_maps.append({"x": xb, "wp": wpacks[hp]})

    trace = os.environ.get("BASS_KERNEL_TRACE", "0") == "1"
    res = run_bass_kernel_spmd(nc, in_maps, core_ids=list(range(N_CORES)),
                               trace=trace)
    if trace:
        kernel.last_exec_time_ns = res.exec_time_ns

    out = np.zeros((B, CH, N), dtype=np.float32)
    for cid in range(N_CORES):
        b, hp, qh = cid // 4, (cid % 4) // 2, cid % 2
        out[b][:, qh * NQ:(qh + 1) * NQ] += res.results[cid]["out"].astype(
            np.float32)
    out += b_out[None, :, None]
    return out.reshape(B, CH, 64, 64)


kernel.last_exec_time_ns = None


# revision 21
# speedup vs baseline: 1.3912x; 1.0170x over previous
"""Trainium2 Bass kernel for spatial multi-head self-attention (dense_transformer).

Module: x[2,256,64,64] -> qkv 1x1 conv -> 4-head attention over n=4096 spatial
positions -> out 1x1 conv + bias.

Sharding (8 cores): core = (batch b, head-pair hp, query-half qh of 2048
positions). Each core computes K/V for ITS 2 heads over the full 4096
positions (half the projection duplication of a batch/q-quarter split),
Q for its 2 heads over its 2048 positions, the full attention + softmax
for its (batch, head-pair, q-half), and the partial output projection
through its heads' w_out rows. The host sums the two head-pair partials
per (batch, q-half) and adds the bias - no device collectives.

Per-core structure, streaming over 32 k-tiles of 128 positions per
q-chunk round (qc of 512, 4 rounds):
  PE : scoresT[k,q] = k_tile.T @ q (the 2 heads row-packed at partitions
       0:64 / 64:128; the two half-row matmuls run CONCURRENTLY via the
       PE's row-group tiling)
  ACT: exp(scores) PSUM->SBUF bf16 (max-subtraction skipped; scores ~N(0,1)
       by construction so exp cannot overflow). A slice of each tile's
       columns is offloaded to the DVE via a Schraudolph bf16 bit-trick exp.
  PE : out += vT_aug.T @ exp_chunk; vT_aug carries a ones column so row 64
       accumulates the softmax denominator for free (stationary padded to
       128 columns for fast weight load; the extra rows are never read).
  DVE/GpSimd: normalize with reciprocal_approx_fast + partition_broadcast
       + gpsimd multiply (SBUF-only, keeps the DVE free for exp).
K-tiles are walked in PAIRS (scores pairs back-to-back, then the attn@V
matmuls chain weight loads through the PE background weight buffer).
PSUM: three rotating 2-bank score slots (shared with projection groups) +
two 1-bank attention accumulators. Projections stream just-in-time inside
round 0 (kproj/vtproj) with the input DMA pieces ordered by need-time;
normalize and the output projection are deferred into the following
round's schedule. Output is written per-qc as bf16 (host casts/sums in
fp32), spread across DMA queues.

Tail: the last round's denominators go through an ACT rowsum evict ->
DVE recip -> PE broadcast-matmul (ones-block stationary [2,128] spreads
rr[2,512] to rb[128,512] in PSUM, replacing two serialized 1us gpsimd
partition_broadcasts) -> DVE multiplies -> 2 outproj matmuls -> dual-queue
bf16 output DMA.

Exp engine split (EXP_SPLIT, tuned on HW): ACT takes ~60-70% of exp
columns, DVE the rest; round 0 gives the DVE less because it also
carries the kproj/vtproj projection evictions.
"""

import os
import sys
import types

import numpy as np

sys.path.insert(0, "/opt/trn_rl_repo")

import ml_dtypes  # noqa: E402

import concourse.bass as bass  # noqa: E402
import concourse.mybir as mybir  # noqa: E402
import concourse.tile as tile  # noqa: E402
from concourse import bacc  # noqa: E402
from concourse.bass_utils import run_bass_kernel_spmd  # noqa: E402

BF16 = mybir.dt.bfloat16
F32 = mybir.dt.float32
I16 = mybir.dt.int16

N_CORES = 8
CH = 256          # x channels
HID = 256         # qkv hidden (4 heads x 64)
H = 4             # heads
DH = 64           # dim per head
N = 4096          # spatial positions (64*64)
NQ = 2048         # query positions per core (q-half)
B = 2             # batch
SCALE = DH ** -0.5
NKT = N // 128    # 32 k-tiles
NQC = NQ // 512   # 4 q-chunks -> 4 rounds

# Schraudolph exp offload. SPLIT[r]: flat column split point S of the
# per-k-tile score block [128, 2*512] (head-major). The ACT exps columns
# [0:S] exactly; the DVE computes [S:1024] with a one-instruction
# Schraudolph bf16 bit-trick exp (rms rel err ~2% on those columns, largely
# cancelled by the shared softmax denominator).
_SP = os.environ.get("EXP_SPLIT", "704,640,640,640").split(",")
SPLIT = {r: int(_SP[r]) for r in range(4)}
LOG2E = float(np.log2(np.e))
SCH_A = 128.0 * LOG2E
SCH_B = 128.0 * (127.0 - 0.043677)


def _install_ntff_hook():
    """The image's antenv lacks axon_hooks; install it so trace=True works."""
    if "antenv.axon_hooks" in sys.modules:
        return
    try:
        mod = types.ModuleType("antenv.axon_hooks")
        mod._hook = None
        mod.set_axon_ntff_profile_hook = lambda h: setattr(mod, "_hook", h)
        mod.get_axon_ntff_profile_hook = lambda: mod._hook
        sys.modules["antenv.axon_hooks"] = mod
        import antenv
        antenv.axon_hooks = mod
        sys.path.insert(0, "/root/.axon_site/trn_agent_boot")
        from trn_boot import _ntff_profile_via_ctypes
        mod.set_axon_ntff_profile_hook(
            _ntff_profile_via_ctypes("/opt/axon/libaxon_pjrt.so")
        )
    except Exception:
        pass


def _build():
    nc = bacc.Bacc("TRN2", target_bir_lowering=False, debug=False,
                   num_devices=N_CORES)

    x_d = nc.dram_tensor("x", [CH, N], BF16, kind="ExternalInput").ap()
    # per-core weight pack [wk0 wk1 wq0 wq1 wv0 wv1 | wo]: this core's
    # head-pair slices only (128 hid columns), 1024 cols total
    wp_d = nc.dram_tensor("wp", [128, 1024], BF16, kind="ExternalInput").ap()
    out_d = nc.dram_tensor("out", [CH, NQ], BF16, kind="ExternalOutput").ap()

    with tile.TileContext(nc) as tc:
        with tc.tile_pool(name="const", bufs=1) as cst, \
             tc.tile_pool(name="scps", bufs=3, space="PSUM") as scps, \
             tc.tile_pool(name="outps", bufs=1, space="PSUM") as outps, \
             tc.tile_pool(name="expb", bufs=9) as expb, \
             tc.tile_pool(name="osb", bufs=2) as osbp, \
             tc.tile_pool(name="ntmp", bufs=2) as ntmp, \
             tc.tile_pool(name="fout", bufs=2) as foutp:

            def proj_ps(shape):
                return scps.tile(shape, F32, name="scp")

            # ---- persistent tensors ----
            wp_sb = cst.tile([128, 1024], BF16, name="wp")
            wk_sb = [wp_sb[:, c * 128:(c + 1) * 128] for c in range(2)]
            wq_sb = [wp_sb[:, 256 + c * 128:256 + (c + 1) * 128] for c in range(2)]
            wv_sb = [wp_sb[:, 512 + c * 128:512 + (c + 1) * 128] for c in range(2)]
            wo_sb = wp_sb[:, 768:1024]          # [128 hc, 256 oc]
            xb0h = [cst.tile([128, 512], BF16, name=f"xb0h{c}") for c in range(2)]
            xbch = [[cst.tile([128, 1024], BF16, name=f"xb{c}_{i}")
                     for i in range(4)] for c in range(2)]
            kch = [cst.tile([128, 512], BF16, name=f"k{n}") for n in range(8)]
            qch = [cst.tile([128, 512], BF16, name=f"q{qc}") for qc in range(NQC)]
            vtt = [cst.tile([128, 2, 128], BF16, name=f"vt{t}")
                   for t in range(NKT)]
            # tail broadcast stationary: head j's recip row lives at partition
            # 32*j (engine partition bases must be 32-aligned); ind maps row 0
            # -> rb cols 0:64, row 32 -> cols 64:128, other rows are zero so
            # the (memset-to-1.0) filler rows of rr contribute nothing
            ind = cst.tile([33, 128], BF16, name="ind")
            rrin = cst.tile([33, 512], F32, name="rrin")
            rrf = cst.tile([33, 512], F32, name="rrf")
            rrb = cst.tile([33, 512], BF16, name="rrb")

            # ---- input DMAs, need-ordered across the three queues ----
            # sync: critical weights first, then x chunk 2, then wo + x31
            nc.sync.dma_start(out=wp_sb[:, 0:512], in_=wp_d[:, 0:512])
            nc.sync.dma_start(out=wp_sb[:, 512:768], in_=wp_d[:, 512:768])
            nc.sync.dma_start(out=xbch[0][2][:], in_=x_d[0:128, 2048:3072])
            nc.sync.dma_start(out=xbch[1][2][:], in_=x_d[128:256, 2048:3072])
            nc.sync.dma_start(out=wp_sb[:, 768:1024], in_=wp_d[:, 768:1024])
            nc.sync.dma_start(out=xbch[1][3][:], in_=x_d[128:256, 3072:4096])
            # gpsimd: tiny ind memsets (feed the warmup matmuls + dummy exp),
            # then its x DMA issues; rrin's big memset rides the idle DVE
            nc.gpsimd.memset(ind[:], 0.0)
            nc.gpsimd.memset(ind[0:1, 0:64], 1.0)
            nc.gpsimd.memset(ind[32:33, 64:128], 1.0)
            nc.vector.memset(rrb[:], 1.0)
            nc.vector.memset(rrin[:], 1.0)
            nc.gpsimd.dma_start(out=xb0h[0][:], in_=x_d[0:128, 0:512])
            nc.gpsimd.dma_start(out=xbch[0][0][:], in_=x_d[0:128, 0:1024])
            nc.gpsimd.dma_start(out=xbch[0][1][:], in_=x_d[0:128, 1024:2048])
            nc.gpsimd.dma_start(out=xbch[0][3][:], in_=x_d[0:128, 3072:4096])
            # scalar: first xb0h piece, then pre-pull the exp ACT table with a
            # dummy activation (so the ~1.3us table load is off the critical
            # path of round 0's first exp), then the rest of its x chunks
            nc.scalar.dma_start(out=xb0h[1][:], in_=x_d[128:256, 0:512])
            nc.scalar.dma_start(out=xbch[1][0][:], in_=x_d[128:256, 0:1024])
            nc.scalar.activation(rrf[0:1, 0:1], ind[0:1, 0:1],
                                 mybir.ActivationFunctionType.Exp)
            nc.scalar.dma_start(out=xbch[1][1][:], in_=x_d[128:256, 1024:2048])
            # PE warmup: a chain of junk matmuls on already-initialized tiles
            # keeps the HAM activity window busy while the input DMA streams,
            # so the first real projections run at 2.4GHz instead of 1.2
            wps = scps.tile([128, 512], F32, name="scp")
            for _ in range(9):
                nc.tensor.matmul(wps[:, 0:256], lhsT=ind[:, :],
                                 rhs=rrb[:, 0:256], start=True, stop=True)

            # ---- projection emitters ----
            def kproj(n):
                ps = proj_ps([128, 512])
                for c in range(2):
                    rhs = (xb0h[c][:] if n == 0 else
                           xbch[c][n // 2][:, (n % 2) * 512:(n % 2 + 1) * 512])
                    nc.tensor.matmul(ps[:], lhsT=wk_sb[c][:], rhs=rhs,
                                     start=(c == 0), stop=(c == 1))
                nc.vector.tensor_copy(kch[n][:], ps[:])

            def qproj(qc):
                ps = proj_ps([128, 512])
                for c in range(2):
                    rhs = (xb0h[c][:] if qc == 0 else
                           xbch[c][qc // 2][:, (qc % 2) * 512:(qc % 2 + 1) * 512])
                    nc.tensor.matmul(ps[:], lhsT=wq_sb[c][:], rhs=rhs,
                                     start=(c == 0), stop=(c == 1))
                # qc0's eviction rides the idle ACT so it doesn't queue behind
                # kch[0]'s eviction on the DVE (both gate scores kt=0)
                if qc == 0:
                    nc.scalar.copy(qch[qc][:], ps[:])
                else:
                    nc.vector.tensor_copy(qch[qc][:], ps[:])

            def vtproj2(tp):
                ps = proj_ps([128, 256])
                for u in range(2):
                    t = 2 * tp + u
                    for c in range(2):
                        nc.tensor.matmul(
                            ps[:, u * 128:(u + 1) * 128],
                            lhsT=xbch[c][t // 8][:, (t % 8) * 128:(t % 8 + 1) * 128],
                            rhs=wv_sb[c],
                            start=(c == 0), stop=(c == 1))
                for u in range(2):
                    t = 2 * tp + u
                    nc.gpsimd.memset(vtt[t][:, :, DH:DH + 1], 1.0)
                    if u == 0:
                        nc.scalar.copy(
                            vtt[t][:, :, 0:DH],
                            ps[:, u * 128:(u + 1) * 128].rearrange(
                                "p (h d) -> p h d", d=DH))
                    else:
                        nc.vector.tensor_copy(
                            vtt[t][:, :, 0:DH],
                            ps[:, u * 128:(u + 1) * 128].rearrange(
                                "p (h d) -> p h d", d=DH))

            # ---- interleave schedules: round index -> {kt: [thunks]} ----
            # Round r == q-chunk r. Round 0 carries kproj(1..7) + vtproj JIT
            # (kproj(n) feeds scores kt=4n; vtproj2(tp) feeds attn@V kt=2tp,
            # which fires ~5 k-tiles behind scores). qproj(qc) spread out.
            sched = {r: {} for r in range(4)}
            sched_pre = {r: {} for r in range(4)}

            def add(r, kt, fn, *a):
                sched[r].setdefault(kt, []).append((fn, a))

            for n in range(1, 8):
                add(0, max(2, 4 * n - 3), kproj, n)
            for tp in range(NKT // 2):
                kt = 2 * tp + 2 if tp < 14 else (29 if tp == 14 else 30)
                add(0, kt, vtproj2, tp)
            add(0, 12, qproj, 1)
            add(1, 4, qproj, 2)
            add(2, 4, qproj, 3)

            # ---- deferred finishers (normalize / out-projection) ----
            o_tiles = {}   # qc -> [128, 512] bf16 (both heads stacked)
            norm_state = {}

            def get_o(qc):
                if qc not in o_tiles:
                    o_tiles[qc] = osbp.tile([128, 512], BF16, name=f"o{qc}")
                return o_tiles[qc]

            def norm_step(ops_j, qc, j, step):
                key = (qc, j)
                if step == 0:
                    rs = ntmp.tile([1, 512], F32, name=f"rs{j}")
                    nc.vector.tensor_copy(rs[:], ops_j[DH:DH + 1, :])
                    un = ntmp.tile([64, 512], F32, name=f"un{j}")
                    nc.vector.tensor_copy(un[:], ops_j[0:DH, :])
                    norm_state[key] = (un, rs)
                elif step == 1:
                    un, rs = norm_state[key]
                    rr = ntmp.tile([1, 512], F32, name=f"rr{j}")
                    nc.vector.reciprocal_approx_fast(out=rr[:], in_=rs[:])
                    rb = ntmp.tile([64, 512], F32, name=f"rb{j}")
                    nc.gpsimd.partition_broadcast(rb[:], rr[:])
                    norm_state[key] = (un, rb)
                else:
                    # DVE, not gpsimd: a gpsimd tensor op would swap the
                    # ext-isa IRAM away from partition_broadcast (~6.7us
                    # reload each way, measured)
                    un, rb = norm_state[key]
                    o = get_o(qc)
                    nc.vector.tensor_mul(
                        out=o[j * DH:(j + 1) * DH, :], in0=un[:], in1=rb[:])

            def outproj(qc, queue):
                # one matmul per output-channel half: contraction is the full
                # 128 hc of this core's 2 heads (o stacked by partition)
                o = o_tiles[qc]
                for mt in range(2):
                    fps = proj_ps([128, 512])
                    nc.tensor.matmul(fps[:], lhsT=wo_sb[:, mt * 128:(mt + 1) * 128],
                                     rhs=o[:], start=True, stop=True)
                    fo = foutp.tile([128, 512], BF16, name="fo")
                    if mt == 0:
                        nc.scalar.copy(fo[:], fps[:])
                    else:
                        nc.vector.tensor_copy(fo[:], fps[:])
                    queue.dma_start(
                        out=out_d[mt * 128:(mt + 1) * 128,
                                  qc * 512:(qc + 1) * 512],
                        in_=fo[:])

            # ---- attention rounds ----
            def round_(r):
                qc = r
                ops = [outps.tile([128, 512], F32, name=f"ops{j}")
                       for j in range(2)]
                S = SPLIT[r]
                pending = []

                def emit_out(kt, eb):
                    for j in range(2):
                        nc.tensor.matmul(
                            ops[j][:],
                            lhsT=vtt[kt][:, j, :],
                            rhs=eb[:, j * 512:(j + 1) * 512],
                            start=(kt == 0), stop=(kt == NKT - 1))

                for kt2 in range(0, NKT, 2):
                    for kt in (kt2, kt2 + 1):
                        for fn, a in sched_pre[r].get(kt, []):
                            fn(*a)
                        scp = scps.tile([128, 1024], F32, name="scp")
                        for j in range(2):
                            nc.tensor.matmul(
                                scp[:, j * 512:(j + 1) * 512],
                                lhsT=kch[kt // 4][
                                    j * 64:(j + 1) * 64,
                                    (kt % 4) * 128:(kt % 4 + 1) * 128],
                                rhs=qch[qc][j * 64:(j + 1) * 64, :],
                                start=True, stop=True)
                        for fn, a in sched[r].get(kt, []):
                            fn(*a)
                        eb = expb.tile([128, 1024], BF16, name="eb")
                        if S > 0:
                            nc.scalar.activation(
                                eb[:, 0:S], scp[:, 0:S],
                                mybir.ActivationFunctionType.Exp)
                        if S < 1024:
                            nc.vector.tensor_scalar(
                                eb[:, S:1024].bitcast(I16), scp[:, S:1024],
                                SCH_A, SCH_B,
                                mybir.AluOpType.mult, mybir.AluOpType.add)
                        pending.append((kt, eb))
                    while len(pending) > 5:
                        emit_out(*pending.pop(0))
                for it in pending:
                    emit_out(*it)
                return ops

            # ---- pre-round projections ----
            kproj(0)
            qproj(0)

            for r in range(4):
                ops = round_(r)
                items = []
                for j in range(2):
                    items += [
                        (1 + j, lambda o=ops[j], q=r, h=j: norm_step(o, q, h, 0)),
                        (4 + 2 * j, lambda q=r, h=j: norm_step(None, q, h, 1)),
                        (8 + 2 * j, lambda q=r, h=j: norm_step(None, q, h, 2)),
                    ]
                # sync is idle mid-kernel; keep output DMA issue off the
                # exp-saturated ACT engine entirely
                qdma = [nc.sync, nc.gpsimd, nc.sync][r % 3]
                items += [(16, lambda q=r, qd=qdma: outproj(q, qd))]
                if r < 3:
                    for kt, fn in items:
                        sched[r + 1].setdefault(kt, []).append((fn, ()))
                else:
                    # ---- tail: minimal-latency normalize for the last round.
                    # DVE recip straight from the PSUM rowsum rows -> bf16
                    # cast -> PE broadcast-matmul (ind spreads rr[2,512] to
                    # rb[128,512] in one N=512 matmul) -> multiplies ->
                    # outproj -> dual-queue output DMA. ACT carries the un
                    # evictions in parallel; junk keepwarm matmuls keep the
                    # PE at 2.4GHz through the chain.
                    for _ in range(8):
                        nc.tensor.matmul(wps[:, 0:128], lhsT=ind[:, :],
                                         rhs=rrb[:, 0:128], start=True,
                                         stop=True)
                    # rowsum rows to partitions 0/32 of rrin, split across the
                    # two PSUM-capable engines so they land in parallel
                    nc.scalar.copy(rrin[0:1, :], ops[0][DH:DH + 1, :])
                    nc.vector.tensor_copy(rrin[32:33, :], ops[1][DH:DH + 1, :])
                    uns = []
                    for j in range(2):
                        un = ntmp.tile([64, 512], F32, name=f"tun{j}")
                        nc.scalar.copy(un[:], ops[j][0:DH, :])
                        uns.append(un)
                    nc.vector.reciprocal_approx_fast(out=rrf[:], in_=rrin[:])
                    nc.vector.tensor_copy(rrb[:], rrf[:])
                    rbps = proj_ps([128, 512])
                    nc.tensor.matmul(rbps[:], lhsT=ind[:], rhs=rrb[:],
                                     start=True, stop=True)
                    o = get_o(3)
                    for j in range(2):
                        nc.vector.tensor_mul(
                            out=o[j * DH:(j + 1) * DH, :], in0=uns[j][:],
                            in1=rbps[j * DH:(j + 1) * DH, :])
                    for mt in range(2):
                        fps = proj_ps([128, 512])
                        nc.tensor.matmul(
                            fps[:], lhsT=wo_sb[:, mt * 128:(mt + 1) * 128],
                            rhs=o[:], start=True, stop=True)
                        fo = foutp.tile([128, 512], BF16, name="fo")
                        if mt == 0:
                            nc.scalar.copy(fo[:], fps[:])
                        else:
                            nc.vector.tensor_copy(fo[:], fps[:])
                        qd = nc.sync if mt == 0 else nc.gpsimd
                        qd.dma_start(
                            out=out_d[mt * 128:(mt + 1) * 128,
                                      3 * 512:4 * 512],
                            in_=fo[:])

    nc.compile()
    return nc


_NC = None


def _get_nc():
    global _NC
    if _NC is None:
        _NC = _build()
    return _NC


def kernel(x, w_qkv, w_out, b_out):
    """Full inputs -> full output, distributed over 8 NeuronCores."""
    _install_ntff_hook()
    nc = _get_nc()

    x = np.asarray(x, dtype=np.float32)
    w_qkv = np.asarray(w_qkv, dtype=np.float32)
    w_out = np.asarray(w_out, dtype=np.float32)
    b_out = np.asarray(b_out, dtype=np.float32)

    bf = ml_dtypes.bfloat16
    xf = x.reshape(B, CH, N)
    # fold the softmax scale into w_q (in fp32, before the bf16 cast)
    wq_t = np.ascontiguousarray((w_qkv[0:HID] * SCALE).T)       # [ch, hid]
    wk_t = np.ascontiguousarray(w_qkv[HID:2 * HID].T)
    wv_t = np.ascontiguousarray(w_qkv[2 * HID:3 * HID].T)
    wo_t = np.ascontiguousarray(w_out.T)                        # [hc, oc]

    wpacks = []
    for hp in range(2):
        s = slice(hp * 128, (hp + 1) * 128)
        wp = np.concatenate(
            [wk_t[0:128, s], wk_t[128:256, s],
             wq_t[0:128, s], wq_t[128:256, s],
             wv_t[0:128, s], wv_t[128:256, s],
             wo_t[s, :]], axis=1)
        wpacks.append(np.ascontiguousarray(wp).astype(bf))

    in_maps = []
    for cid in range(N_CORES):
        b, hp, qh = cid // 4, (cid % 4) // 2, cid % 2
        # rotate the position chunks so chunks 0-1 are this core's q-half
        # (softmax over k positions is permutation-invariant)
        perm = [2 * qh, 2 * qh + 1] + [i for i in range(4)
                                       if i not in (2 * qh, 2 * qh + 1)]
        xb = np.ascontiguousarray(
            xf[b].reshape(CH, 4, 1024)[:, perm, :].reshape(CH, N)).astype(bf)
        in_maps.append({"x": xb, "wp": wpacks[hp]})

    trace = os.environ.get("BASS_KERNEL_TRACE", "0") == "1"
    res = run_bass_kernel_spmd(nc, in_maps, core_ids=list(range(N_CORES)),
                               trace=trace)
    if trace:
        kernel.last_exec_time_ns = res.exec_time_ns

    out = np.zeros((B, CH, N), dtype=np.float32)
    for cid in range(N_CORES):
        b, hp, qh = cid // 4, (cid % 4) // 2, cid % 2
        out[b][:, qh * NQ:(qh + 1) * NQ] += res.results[cid]["out"].astype(
            np.float32)
    out += b_out[None, :, None]
    return out.reshape(B, CH, 64, 64)


kernel.last_exec_time_ns = None
